# revision 9
# baseline (speedup 1.0000x reference)
"""Trainium2 Bass kernel for nn_MinigridPPOLSTMAgent.

Data-parallel over envs: B=256 split across 8 NeuronCores (32 envs each).
Per-core layout keeps features on SBUF partitions and images on the free
dim end-to-end:
  - obs tiles are PE-transposed into per-image-row feature chunks
  - conv1/2/3 are block-Toeplitz matmuls (host-built, zero-embedded)
  - the LSTM input projection is 8 accumulating K=128 matmuls per gate
  - the recurrence runs with gate weights as the stationary operand over
    [128 hid, 32 env] tiles; masks are PE-broadcast from the done flags
  - actor/critic heads + final transposes run on the PE
"""
import sys

sys.path.insert(0, '/opt/trn_rl_repo')

import numpy as np
import concourse.bass as bass
import concourse.mybir as mybir
import concourse.tile as tile
from concourse import bacc
from concourse.bass_utils import run_bass_kernel_spmd

T, B, HID, NA = 128, 256, 128, 7
M = 8
BL = B // M          # 32 envs per core
N_LOC = T * BL       # 4096 images per core
NBLK = 256           # images per bulk block (= 8 timesteps)
NBLOCKS = N_LOC // NBLK
F32 = mybir.dt.float32
AF = mybir.ActivationFunctionType


# ---------------------------------------------------------------- program
def build_program():
    nc = bacc.Bacc("TRN2", target_bir_lowering=False, debug=False)

    def din(name, shape):
        return nc.dram_tensor(name, shape, F32, kind="ExternalInput")

    def dout(name, shape):
        return nc.dram_tensor(name, shape, F32, kind="ExternalOutput")

    obs_d = din("obs_c", [N_LOC, 147])
    done_d = din("doneT", [1, N_LOC])
    h0_d = din("h0T", [128, BL])
    c0_d = din("c0T", [128, BL])
    eye_d = din("eye", [128, 128])
    ones_d = din("ones1", [1, 128])
    t1_d = din("T1", [42, 96])
    t2_d = din("T2f", [96, 4 * 80])       # (dy,h) major on cols
    t3_d = din("T3f", [80, 8 * 128])      # ((dy,h,m)) major on cols
    wih_d = din("WIH", [128, 8 * 512])    # k major on cols
    whh_d = din("WHH", [128, 512])
    bias_d = din("BIAS", [128, 4])
    w8_d = din("W8", [128, 8])
    b8_d = din("B8", [8, 1])
    cb1_d = din("CB1", [96, 1])
    cb2_d = din("CB2", [80, 2])
    cb3_d = din("CB3", [128, 2])

    out8_d = dout("out8", [N_LOC, 8])
    ht_d = dout("hT_o", [BL, 128])
    ct_d = dout("cT_o", [BL, 128])

    from contextlib import ExitStack
    with tile.TileContext(nc) as tc, ExitStack() as es:
        sing = es.enter_context(tc.tile_pool(name="sing", bufs=1))
        big = es.enter_context(tc.tile_pool(name="big", bufs=1))
        obs_p = es.enter_context(tc.tile_pool(name="obs", bufs=4))
        x0_p = es.enter_context(tc.tile_pool(name="x0", bufs=8))
        x1_p = es.enter_context(tc.tile_pool(name="x1", bufs=2))
        x2_p = es.enter_context(tc.tile_pool(name="x2", bufs=2))
        x3_p = es.enter_context(tc.tile_pool(name="x3", bufs=2))
        rec_p = es.enter_context(tc.tile_pool(name="rec", bufs=3))
        cpool = es.enter_context(tc.tile_pool(name="cst", bufs=2))
        o8_p = es.enter_context(tc.tile_pool(name="o8", bufs=2))
        ps = es.enter_context(tc.tile_pool(name="ps", bufs=6, space="PSUM"))
        psg = es.enter_context(tc.tile_pool(name="psg", bufs=2, space="PSUM"))

        def load(dram, shape):
            t = sing.tile(shape, F32, tag=dram.name)
            nc.sync.dma_start(t[:], dram[:])
            return t

        eye = load(eye_d, [128, 128])
        ones1 = load(ones_d, [1, 128])
        t1 = load(t1_d, [42, 96])
        t2 = load(t2_d, [96, 320])
        t3 = load(t3_d, [80, 1024])
        wih = load(wih_d, [128, 4096])
        whh = load(whh_d, [128, 512])
        bias = load(bias_d, [128, 4])
        w8 = load(w8_d, [128, 8])
        b8 = load(b8_d, [8, 1])
        cb1 = load(cb1_d, [96, 1])
        cb2 = load(cb2_d, [80, 2])
        cb3 = load(cb3_d, [128, 2])
        done_s = load(done_d, [1, N_LOC])
        h0s = load(h0_d, [128, BL])
        c0s = load(c0_d, [128, BL])

        gx = big.tile([128, 128 * N_LOC // BL], F32, tag="gx")   # [128, 16384]
        msk = big.tile([128, N_LOC], F32, tag="msk")
        hbuf = big.tile([128, N_LOC], F32, tag="hbuf")

        # ---------------- bulk phase: convs + x-projection + masks
        for blk in range(NBLOCKS):
            i0 = blk * NBLK
            obs_t = []
            for half in range(2):
                ot = obs_p.tile([128, 147], F32, tag="obs")
                nc.sync.dma_start(
                    ot[:], obs_d[i0 + 128 * half: i0 + 128 * (half + 1), :])
                obs_t.append(ot)
            # transpose into per-y1 feature chunks
            x0t = []
            for y1 in range(6):
                xt = x0_p.tile([42, NBLK], F32, tag="x0")
                for half in range(2):
                    pt = ps.tile([42, 128], F32, tag="bank")
                    nc.tensor.transpose(
                        pt[:], obs_t[half][:, 21 * y1: 21 * y1 + 42], eye[:])
                    nc.vector.tensor_copy(
                        xt[:, 128 * half: 128 * (half + 1)], pt[:])
                x0t.append(xt)
            # conv1
            x1t = x1_p.tile([96, 6 * NBLK], F32, tag="x1")
            for y1 in range(6):
                p1 = ps.tile([96, NBLK], F32, tag="bank")
                nc.tensor.matmul(p1[:], t1[:], x0t[y1][:], start=True, stop=True)
                nc.scalar.activation(
                    x1t[:, NBLK * y1: NBLK * (y1 + 1)], p1[:], AF.Relu,
                    bias=cb1[:, 0:1])
            # conv2  (x2 col layout: (h,y2) blocks)
            x2t = x2_p.tile([80, 10 * NBLK], F32, tag="x2")
            for y2 in range(5):
                for h in range(2):
                    p2 = ps.tile([80, NBLK], F32, tag="bank")
                    for dy in range(2):
                        nc.tensor.matmul(
                            p2[:],
                            t2[:, 80 * (dy * 2 + h): 80 * (dy * 2 + h + 1)],
                            x1t[:, NBLK * (y2 + dy): NBLK * (y2 + dy + 1)],
                            start=(dy == 0), stop=(dy == 1))
                    c = h * 5 + y2
                    nc.scalar.activation(
                        x2t[:, NBLK * c: NBLK * (c + 1)], p2[:], AF.Relu,
                        bias=cb2[:, h: h + 1])
            # conv3 (x3 chunk k = y3*2+m)
            x3t = x3_p.tile([128, 8 * NBLK], F32, tag="x3")
            for y3 in range(4):
                for mm in range(2):
                    p3 = ps.tile([128, NBLK], F32, tag="bank")
                    i = 0
                    for dy in range(2):
                        for h in range(2):
                            w = (dy * 2 + h) * 2 + mm
                            c = h * 5 + (y3 + dy)
                            nc.tensor.matmul(
                                p3[:],
                                t3[:, 128 * w: 128 * (w + 1)],
                                x2t[:, NBLK * c: NBLK * (c + 1)],
                                start=(i == 0), stop=(i == 3))
                            i += 1
                    k = y3 * 2 + mm
                    nc.scalar.activation(
                        x3t[:, NBLK * k: NBLK * (k + 1)], p3[:], AF.Relu,
                        bias=cb3[:, mm: mm + 1])
            # x-projection per gate (gate cols order: i,f,o,g)
            gxv = gx[:].rearrange("p (t g b) -> p t g b", g=4, b=BL)
            for g in range(4):
                px = ps.tile([128, NBLK], F32, tag="bank")
                for k in range(8):
                    nc.tensor.matmul(
                        px[:],
                        wih[:, 512 * k + 128 * g: 512 * k + 128 * (g + 1)],
                        x3t[:, NBLK * k: NBLK * (k + 1)],
                        start=(k == 0), stop=(k == 7))
                dest = gxv[:, 8 * blk: 8 * (blk + 1), g: g + 1, :]
                src = px[:].rearrange("p (t b) -> p t b", b=BL)[:, :, None, :]
                nc.scalar.activation(dest, src, AF.Identity,
                                     bias=bias[:, g: g + 1])
            # masks: broadcast (1-done) across partitions
            pm = ps.tile([128, NBLK], F32, tag="bank")
            nc.tensor.matmul(pm[:], ones1[:], done_s[0:1, i0: i0 + NBLK],
                             start=True, stop=True)
            nc.scalar.activation(msk[:, i0: i0 + NBLK], pm[:], AF.Identity,
                                 scale=-1.0, bias=1.0)

        # ---------------- recurrence
        h_prev = h0s[:]
        c_prev = c0s[:]
        for t in range(T):
            mt = msk[:, BL * t: BL * (t + 1)]
            hm = rec_p.tile([128, BL], F32, tag="hm")
            nc.vector.tensor_mul(hm[:], h_prev, mt)
            gp = psg.tile([128, 128], F32, tag="gp")
            for g in range(4):
                nc.tensor.matmul(
                    gp[:, BL * g: BL * (g + 1)],
                    whh[:, 128 * g: 128 * (g + 1)], hm[:],
                    start=True, stop=True)
            gt = rec_p.tile([128, 128], F32, tag="gt")
            nc.vector.tensor_add(gt[:], gp[:], gx[:, 128 * t: 128 * (t + 1)])
            sg = rec_p.tile([128, 128], F32, tag="sg")
            nc.scalar.activation(sg[:, 0:96], gt[:, 0:96], AF.Sigmoid)
            nc.scalar.activation(sg[:, 96:128], gt[:, 96:128], AF.Tanh)
            cm = rec_p.tile([128, BL], F32, tag="cm")
            nc.vector.tensor_mul(cm[:], c_prev, mt)
            fc = rec_p.tile([128, BL], F32, tag="fc")
            nc.vector.tensor_mul(fc[:], sg[:, 32:64], cm[:])
            ig = rec_p.tile([128, BL], F32, tag="ig")
            nc.vector.tensor_mul(ig[:], sg[:, 0:32], sg[:, 96:128])
            cn = cpool.tile([128, BL], F32, tag="c")
            nc.vector.tensor_add(cn[:], fc[:], ig[:])
            tcv = rec_p.tile([128, BL], F32, tag="tc")
            nc.scalar.activation(tcv[:], cn[:], AF.Tanh)
            nc.vector.tensor_mul(hbuf[:, BL * t: BL * (t + 1)],
                                 sg[:, 64:96], tcv[:])
            h_prev = hbuf[:, BL * t: BL * (t + 1)]
            c_prev = cn[:]

        # ---------------- heads + output transposes
        for j in range(8):
            p8 = ps.tile([8, 512], F32, tag="bank")
            nc.tensor.matmul(p8[:], w8[:], hbuf[:, 512 * j: 512 * (j + 1)],
                             start=True, stop=True)
            o8t = o8_p.tile([8, 512], F32, tag="o8")
            nc.scalar.activation(o8t[:], p8[:], AF.Identity, bias=b8[:, 0:1])
            for q in range(4):
                pt8 = ps.tile([128, 8], F32, tag="bank")
                nc.tensor.transpose(pt8[:], o8t[:, 128 * q: 128 * (q + 1)],
                                    eye[0:8, 0:8])
                ot8 = o8_p.tile([128, 8], F32, tag="ot8")
                nc.vector.tensor_copy(ot8[:], pt8[:])
                r0 = 512 * j + 128 * q
                nc.sync.dma_start(out8_d[r0: r0 + 128, :], ot8[:])
        # final h/c (transposed to [BL, 128])
        for src, dst in ((h_prev, ht_d), (c_prev, ct_d)):
            phc = ps.tile([BL, 128], F32, tag="bank")
            nc.tensor.transpose(phc[:], src, eye[:])
            hct = o8_p.tile([BL, 128], F32, tag="hct")
            nc.vector.tensor_copy(hct[:], phc[:])
            nc.sync.dma_start(dst[:], hct[:])

    nc.compile()
    return nc


# ---------------------------------------------------------------- host side
def build_toeplitz(conv1_w, conv2_w, conv3_w):
    T1 = np.zeros((42, 96), np.float32)
    for c1 in range(16):
        for x1 in range(6):
            for dy in range(2):
                for dx in range(2):
                    for c in range(3):
                        T1[21 * dy + 3 * (x1 + dx) + c, c1 * 6 + x1] = \
                            conv1_w[c1, c, dy, dx]
    T2 = np.zeros((2, 2, 96, 80), np.float32)
    for h in range(2):
        for c2p in range(16):
            for x2 in range(5):
                for dy in range(2):
                    for dx in range(2):
                        for c1 in range(16):
                            T2[dy, h, c1 * 6 + (x2 + dx), c2p * 5 + x2] = \
                                conv2_w[16 * h + c2p, c1, dy, dx]
    T3 = np.zeros((2, 2, 2, 80, 128), np.float32)
    for mm in range(2):
        for c3p in range(32):
            for x3 in range(4):
                for dy in range(2):
                    for dx in range(2):
                        for h in range(2):
                            for c2p in range(16):
                                T3[dy, h, mm, c2p * 5 + (x3 + dx), c3p * 4 + x3] = \
                                    conv3_w[32 * mm + c3p, 16 * h + c2p, dy, dx]
    # flatten to the on-device column layouts
    T2f = np.zeros((96, 4 * 80), np.float32)
    for dy in range(2):
        for h in range(2):
            T2f[:, 80 * (dy * 2 + h): 80 * (dy * 2 + h + 1)] = T2[dy, h]
    T3f = np.zeros((80, 8 * 128), np.float32)
    for dy in range(2):
        for h in range(2):
            for mm in range(2):
                w = (dy * 2 + h) * 2 + mm
                T3f[:, 128 * w: 128 * (w + 1)] = T3[dy, h, mm]
    return T1, T2f, T3f


def build_lstm_weights(w_ih, w_hh, b_ih, b_hh, actor_w, actor_b,
                       critic_w, critic_b):
    gperm = np.concatenate([np.arange(0, 128), np.arange(128, 256),
                            np.arange(384, 512), np.arange(256, 384)])
    fperm = np.zeros(1024, np.int64)
    idx = 0
    for y3 in range(4):
        for mm in range(2):
            for c3p in range(32):
                for x3 in range(4):
                    fperm[idx] = (c3p + 32 * mm) * 16 + y3 * 4 + x3
                    idx += 1
    wp = w_ih[gperm][:, fperm].T            # [1024, 512]
    WIH = np.zeros((128, 8 * 512), np.float32)
    for k in range(8):
        WIH[:, 512 * k: 512 * (k + 1)] = wp[128 * k: 128 * (k + 1)]
    WHH = np.ascontiguousarray(w_hh[gperm].T, np.float32)
    BIAS = np.ascontiguousarray((b_ih + b_hh)[gperm].reshape(4, 128).T,
                                np.float32)
    W8 = np.ascontiguousarray(np.concatenate([actor_w, critic_w], 0).T,
                              np.float32)
    B8 = np.ascontiguousarray(
        np.concatenate([actor_b, critic_b])[:, None], np.float32)
    return WIH, WHH, BIAS, W8, B8


_NC = None


def kernel(obs, done, h0, c0,
           conv1_w, conv1_b, conv2_w, conv2_b, conv3_w, conv3_b,
           w_ih, w_hh, b_ih, b_hh, actor_w, actor_b, critic_w, critic_b):
    global _NC
    args = [np.asarray(a, np.float32) for a in
            (obs, done, h0, c0, conv1_w, conv1_b, conv2_w, conv2_b,
             conv3_w, conv3_b, w_ih, w_hh, b_ih, b_hh, actor_w, actor_b,
             critic_w, critic_b)]
    (obs, done, h0, c0, conv1_w, conv1_b, conv2_w, conv2_b, conv3_w,
     conv3_b, w_ih, w_hh, b_ih, b_hh, actor_w, actor_b, critic_w,
     critic_b) = args

    if _NC is None:
        _NC = build_program()
    nc = _NC

    T1, T2f, T3f = build_toeplitz(conv1_w, conv2_w, conv3_w)
    WIH, WHH, BIAS, W8, B8 = build_lstm_weights(
        w_ih, w_hh, b_ih, b_hh, actor_w, actor_b, critic_w, critic_b)
    # conv biases are per-out-channel -> per-partition vectors in the
    # (channel, x) partition layouts used on device.
    CB1 = np.repeat(conv1_b, 6)[:, None].astype(np.float32)          # [96,1]
    CB2 = np.stack([np.repeat(conv2_b[16 * h: 16 * (h + 1)], 5)
                    for h in range(2)], 1).astype(np.float32)        # [80,2]
    CB3 = np.stack([np.repeat(conv3_b[32 * m: 32 * (m + 1)], 4)
                    for m in range(2)], 1).astype(np.float32)        # [128,2]
    shared = dict(eye=np.eye(128, dtype=np.float32),
                  ones1=np.ones((1, 128), np.float32),
                  T1=T1, T2f=T2f, T3f=T3f, WIH=WIH, WHH=WHH, BIAS=BIAS,
                  W8=W8, B8=B8, CB1=CB1, CB2=CB2, CB3=CB3)

    obs_r = obs.reshape(T, B, 147)
    done_r = done.reshape(T, B)
    in_maps = []
    for m in range(M):
        sl = slice(m * BL, (m + 1) * BL)
        in_maps.append(dict(
            obs_c=np.ascontiguousarray(obs_r[:, sl].reshape(N_LOC, 147)),
            doneT=np.ascontiguousarray(done_r[:, sl].reshape(1, N_LOC)),
            h0T=np.ascontiguousarray(h0[0, sl].T),
            c0T=np.ascontiguousarray(c0[0, sl].T),
            **shared))

    res = run_bass_kernel_spmd(nc, in_maps, list(range(M)))

    out = np.zeros((T, B, 8), np.float32)
    hT = np.zeros((1, B, 128), np.float32)
    cT = np.zeros((1, B, 128), np.float32)
    for m in range(M):
        sl = slice(m * BL, (m + 1) * BL)
        out[:, sl] = res.results[m]["out8"].reshape(T, BL, 8)
        hT[0, sl] = res.results[m]["hT_o"]
        cT[0, sl] = res.results[m]["cT_o"]
    return out.reshape(T * B, 8), hT, cT


# revision 16
# speedup vs baseline: 64.4397x; 64.4397x over previous
"""Trainium2 Bass kernel for nn_MinigridPPOLSTMAgent.

Data-parallel over envs: B=256 split across 8 NeuronCores (32 envs each).
Per-core layout keeps features on SBUF partitions and images on the free
dim end-to-end:
  - obs tiles are PE-transposed into per-image-row feature chunks
  - conv1/2/3 are block-Toeplitz matmuls (host-built, zero-embedded)
  - the LSTM input projection is 8 accumulating K=128 matmuls per gate
  - the recurrence runs with gate weights as the stationary operand over
    [128 hid, 32 env] tiles; masks are PE-broadcast from the done flags
  - actor/critic heads + final transposes run on the PE
"""
import sys

sys.path.insert(0, '/opt/trn_rl_repo')

import numpy as np
import concourse.bass as bass
import concourse.mybir as mybir
import concourse.tile as tile
from concourse import bacc

T, B, HID, NA = 128, 256, 128, 7
M = 8
BL = B // M          # 32 envs per core
N_LOC = T * BL       # 4096 images per core
NBLK = 256           # images per bulk block (= 8 timesteps)
NBLOCKS = N_LOC // NBLK
F32 = mybir.dt.float32
AF = mybir.ActivationFunctionType


# ---------------------------------------------------------------- program
def build_program():
    nc = bacc.Bacc("TRN2", target_bir_lowering=False, debug=False)

    def din(name, shape):
        return nc.dram_tensor(name, shape, F32, kind="ExternalInput")

    def dout(name, shape):
        return nc.dram_tensor(name, shape, F32, kind="ExternalOutput")

    obs_d = din("obs_c", [N_LOC, 147])
    done_d = din("doneT", [1, N_LOC])
    h0_d = din("h0T", [128, BL])
    c0_d = din("c0T", [128, BL])
    eye_d = din("eye", [128, 128])
    ones_d = din("ones1", [1, 128])
    t1_d = din("T1", [42, 96])
    t2_d = din("T2f", [96, 4 * 80])       # (dy,h) major on cols
    t3_d = din("T3f", [80, 8 * 128])      # ((dy,h,m)) major on cols
    wih_d = din("WIH", [128, 8 * 512])    # k major on cols
    whh_d = din("WHH", [128, 512])
    bias_d = din("BIAS", [128, 4])
    w8_d = din("W8", [128, 8])
    b8_d = din("B8", [8, 1])
    cb1_d = din("CB1", [96, 1])
    cb2_d = din("CB2", [80, 2])
    cb3_d = din("CB3", [128, 2])

    out8_d = dout("out8", [N_LOC, 8])
    ht_d = dout("hT_o", [BL, 128])
    ct_d = dout("cT_o", [BL, 128])

    from contextlib import ExitStack
    with tile.TileContext(nc) as tc, ExitStack() as es:
        sing = es.enter_context(tc.tile_pool(name="sing", bufs=1))
        big = es.enter_context(tc.tile_pool(name="big", bufs=1))
        obs_p = es.enter_context(tc.tile_pool(name="obs", bufs=4))
        x0_p = es.enter_context(tc.tile_pool(name="x0", bufs=8))
        x1_p = es.enter_context(tc.tile_pool(name="x1", bufs=2))
        x2_p = es.enter_context(tc.tile_pool(name="x2", bufs=2))
        x3_p = es.enter_context(tc.tile_pool(name="x3", bufs=2))
        rec_p = es.enter_context(tc.tile_pool(name="rec", bufs=3))
        cpool = es.enter_context(tc.tile_pool(name="cst", bufs=2))
        o8_p = es.enter_context(tc.tile_pool(name="o8", bufs=2))
        ps = es.enter_context(tc.tile_pool(name="ps", bufs=6, space="PSUM"))
        psg = es.enter_context(tc.tile_pool(name="psg", bufs=2, space="PSUM"))

        def load(dram, shape):
            t = sing.tile(shape, F32, tag=dram.name)
            nc.sync.dma_start(t[:], dram[:])
            return t

        eye = load(eye_d, [128, 128])
        ones1 = load(ones_d, [1, 128])
        t1 = load(t1_d, [42, 96])
        t2 = load(t2_d, [96, 320])
        t3 = load(t3_d, [80, 1024])
        wih = load(wih_d, [128, 4096])
        whh = load(whh_d, [128, 512])
        bias = load(bias_d, [128, 4])
        w8 = load(w8_d, [128, 8])
        b8 = load(b8_d, [8, 1])
        cb1 = load(cb1_d, [96, 1])
        cb2 = load(cb2_d, [80, 2])
        cb3 = load(cb3_d, [128, 2])
        done_s = load(done_d, [1, N_LOC])
        h0s = load(h0_d, [128, BL])
        c0s = load(c0_d, [128, BL])

        gx = big.tile([128, 128 * N_LOC // BL], F32, tag="gx")   # [128, 16384]
        msk = big.tile([128, N_LOC], F32, tag="msk")
        hbuf = big.tile([128, N_LOC], F32, tag="hbuf")

        # ---------------- bulk phase: convs + x-projection + masks
        for blk in range(NBLOCKS):
            i0 = blk * NBLK
            obs_t = []
            for half in range(2):
                ot = obs_p.tile([128, 147], F32, tag="obs")
                nc.sync.dma_start(
                    ot[:], obs_d[i0 + 128 * half: i0 + 128 * (half + 1), :])
                obs_t.append(ot)
            # transpose into per-y1 feature chunks
            x0t = []
            for y1 in range(6):
                xt = x0_p.tile([42, NBLK], F32, tag="x0")
                for half in range(2):
                    pt = ps.tile([42, 128], F32, tag="bank")
                    nc.tensor.transpose(
                        pt[:], obs_t[half][:, 21 * y1: 21 * y1 + 42], eye[:])
                    nc.vector.tensor_copy(
                        xt[:, 128 * half: 128 * (half + 1)], pt[:])
                x0t.append(xt)
            # conv1
            x1t = x1_p.tile([96, 6 * NBLK], F32, tag="x1")
            for y1 in range(6):
                p1 = ps.tile([96, NBLK], F32, tag="bank")
                nc.tensor.matmul(p1[:], t1[:], x0t[y1][:], start=True, stop=True)
                nc.scalar.activation(
                    x1t[:, NBLK * y1: NBLK * (y1 + 1)], p1[:], AF.Relu,
                    bias=cb1[:, 0:1])
            # conv2  (x2 col layout: (h,y2) blocks)
            x2t = x2_p.tile([80, 10 * NBLK], F32, tag="x2")
            for y2 in range(5):
                for h in range(2):
                    p2 = ps.tile([80, NBLK], F32, tag="bank")
                    for dy in range(2):
                        nc.tensor.matmul(
                            p2[:],
                            t2[:, 80 * (dy * 2 + h): 80 * (dy * 2 + h + 1)],
                            x1t[:, NBLK * (y2 + dy): NBLK * (y2 + dy + 1)],
                            start=(dy == 0), stop=(dy == 1))
                    c = h * 5 + y2
                    nc.scalar.activation(
                        x2t[:, NBLK * c: NBLK * (c + 1)], p2[:], AF.Relu,
                        bias=cb2[:, h: h + 1])
            # conv3 (x3 chunk k = y3*2+m)
            x3t = x3_p.tile([128, 8 * NBLK], F32, tag="x3")
            for y3 in range(4):
                for mm in range(2):
                    p3 = ps.tile([128, NBLK], F32, tag="bank")
                    i = 0
                    for dy in range(2):
                        for h in range(2):
                            w = (dy * 2 + h) * 2 + mm
                            c = h * 5 + (y3 + dy)
                            nc.tensor.matmul(
                                p3[:],
                                t3[:, 128 * w: 128 * (w + 1)],
                                x2t[:, NBLK * c: NBLK * (c + 1)],
                                start=(i == 0), stop=(i == 3))
                            i += 1
                    k = y3 * 2 + mm
                    nc.scalar.activation(
                        x3t[:, NBLK * k: NBLK * (k + 1)], p3[:], AF.Relu,
                        bias=cb3[:, mm: mm + 1])
            # x-projection per gate (gate cols order: i,f,o,g)
            gxv = gx[:].rearrange("p (t g b) -> p t g b", g=4, b=BL)
            for g in range(4):
                px = ps.tile([128, NBLK], F32, tag="bank")
                for k in range(8):
                    nc.tensor.matmul(
                        px[:],
                        wih[:, 512 * k + 128 * g: 512 * k + 128 * (g + 1)],
                        x3t[:, NBLK * k: NBLK * (k + 1)],
                        start=(k == 0), stop=(k == 7))
                dest = gxv[:, 8 * blk: 8 * (blk + 1), g: g + 1, :]
                src = px[:].rearrange("p (t b) -> p t b", b=BL)[:, :, None, :]
                nc.scalar.activation(dest, src, AF.Identity,
                                     bias=bias[:, g: g + 1])
            # masks: broadcast (1-done) across partitions
            pm = ps.tile([128, NBLK], F32, tag="bank")
            nc.tensor.matmul(pm[:], ones1[:], done_s[0:1, i0: i0 + NBLK],
                             start=True, stop=True)
            nc.scalar.activation(msk[:, i0: i0 + NBLK], pm[:], AF.Identity,
                                 scale=-1.0, bias=1.0)

        # ---------------- recurrence
        h_prev = h0s[:]
        c_prev = c0s[:]
        for t in range(T):
            mt = msk[:, BL * t: BL * (t + 1)]
            hm = rec_p.tile([128, BL], F32, tag="hm")
            nc.vector.tensor_mul(hm[:], h_prev, mt)
            gp = psg.tile([128, 128], F32, tag="gp")
            for g in range(4):
                nc.tensor.matmul(
                    gp[:, BL * g: BL * (g + 1)],
                    whh[:, 128 * g: 128 * (g + 1)], hm[:],
                    start=True, stop=True)
            gt = rec_p.tile([128, 128], F32, tag="gt")
            nc.vector.tensor_add(gt[:], gp[:], gx[:, 128 * t: 128 * (t + 1)])
            sg = rec_p.tile([128, 128], F32, tag="sg")
            nc.scalar.activation(sg[:, 0:96], gt[:, 0:96], AF.Sigmoid)
            nc.scalar.activation(sg[:, 96:128], gt[:, 96:128], AF.Tanh)
            cm = rec_p.tile([128, BL], F32, tag="cm")
            nc.vector.tensor_mul(cm[:], c_prev, mt)
            fc = rec_p.tile([128, BL], F32, tag="fc")
            nc.vector.tensor_mul(fc[:], sg[:, 32:64], cm[:])
            ig = rec_p.tile([128, BL], F32, tag="ig")
            nc.vector.tensor_mul(ig[:], sg[:, 0:32], sg[:, 96:128])
            cn = cpool.tile([128, BL], F32, tag="c")
            nc.vector.tensor_add(cn[:], fc[:], ig[:])
            tcv = rec_p.tile([128, BL], F32, tag="tc")
            nc.scalar.activation(tcv[:], cn[:], AF.Tanh)
            nc.vector.tensor_mul(hbuf[:, BL * t: BL * (t + 1)],
                                 sg[:, 64:96], tcv[:])
            h_prev = hbuf[:, BL * t: BL * (t + 1)]
            c_prev = cn[:]

        # ---------------- heads + output transposes
        for j in range(8):
            p8 = ps.tile([8, 512], F32, tag="bank")
            nc.tensor.matmul(p8[:], w8[:], hbuf[:, 512 * j: 512 * (j + 1)],
                             start=True, stop=True)
            o8t = o8_p.tile([8, 512], F32, tag="o8")
            nc.scalar.activation(o8t[:], p8[:], AF.Identity, bias=b8[:, 0:1])
            for q in range(4):
                pt8 = ps.tile([128, 8], F32, tag="bank")
                nc.tensor.transpose(pt8[:], o8t[:, 128 * q: 128 * (q + 1)],
                                    eye[0:8, 0:8])
                ot8 = o8_p.tile([128, 8], F32, tag="ot8")
                nc.vector.tensor_copy(ot8[:], pt8[:])
                r0 = 512 * j + 128 * q
                nc.sync.dma_start(out8_d[r0: r0 + 128, :], ot8[:])
        # final h/c (transposed to [BL, 128])
        for src, dst in ((h_prev, ht_d), (c_prev, ct_d)):
            phc = ps.tile([BL, 128], F32, tag="bank")
            nc.tensor.transpose(phc[:], src, eye[:])
            hct = o8_p.tile([BL, 128], F32, tag="hct")
            nc.vector.tensor_copy(hct[:], phc[:])
            nc.sync.dma_start(dst[:], hct[:])

    nc.compile()
    return nc


# ---------------------------------------------------------------- host side
def build_toeplitz(conv1_w, conv2_w, conv3_w):
    T1 = np.zeros((42, 96), np.float32)
    for c1 in range(16):
        for x1 in range(6):
            for dy in range(2):
                for dx in range(2):
                    for c in range(3):
                        T1[21 * dy + 3 * (x1 + dx) + c, c1 * 6 + x1] = \
                            conv1_w[c1, c, dy, dx]
    T2 = np.zeros((2, 2, 96, 80), np.float32)
    for h in range(2):
        for c2p in range(16):
            for x2 in range(5):
                for dy in range(2):
                    for dx in range(2):
                        for c1 in range(16):
                            T2[dy, h, c1 * 6 + (x2 + dx), c2p * 5 + x2] = \
                                conv2_w[16 * h + c2p, c1, dy, dx]
    T3 = np.zeros((2, 2, 2, 80, 128), np.float32)
    for mm in range(2):
        for c3p in range(32):
            for x3 in range(4):
                for dy in range(2):
                    for dx in range(2):
                        for h in range(2):
                            for c2p in range(16):
                                T3[dy, h, mm, c2p * 5 + (x3 + dx), c3p * 4 + x3] = \
                                    conv3_w[32 * mm + c3p, 16 * h + c2p, dy, dx]
    # flatten to the on-device column layouts
    T2f = np.zeros((96, 4 * 80), np.float32)
    for dy in range(2):
        for h in range(2):
            T2f[:, 80 * (dy * 2 + h): 80 * (dy * 2 + h + 1)] = T2[dy, h]
    T3f = np.zeros((80, 8 * 128), np.float32)
    for dy in range(2):
        for h in range(2):
            for mm in range(2):
                w = (dy * 2 + h) * 2 + mm
                T3f[:, 128 * w: 128 * (w + 1)] = T3[dy, h, mm]
    return T1, T2f, T3f


def build_lstm_weights(w_ih, w_hh, b_ih, b_hh, actor_w, actor_b,
                       critic_w, critic_b):
    gperm = np.concatenate([np.arange(0, 128), np.arange(128, 256),
                            np.arange(384, 512), np.arange(256, 384)])
    fperm = np.zeros(1024, np.int64)
    idx = 0
    for y3 in range(4):
        for mm in range(2):
            for c3p in range(32):
                for x3 in range(4):
                    fperm[idx] = (c3p + 32 * mm) * 16 + y3 * 4 + x3
                    idx += 1
    wp = w_ih[gperm][:, fperm].T            # [1024, 512]
    WIH = np.zeros((128, 8 * 512), np.float32)
    for k in range(8):
        WIH[:, 512 * k: 512 * (k + 1)] = wp[128 * k: 128 * (k + 1)]
    WHH = np.ascontiguousarray(w_hh[gperm].T, np.float32)
    BIAS = np.ascontiguousarray((b_ih + b_hh)[gperm].reshape(4, 128).T,
                                np.float32)
    W8 = np.ascontiguousarray(np.concatenate([actor_w, critic_w], 0).T,
                              np.float32)
    B8 = np.ascontiguousarray(
        np.concatenate([actor_b, critic_b])[:, None], np.float32)
    return WIH, WHH, BIAS, W8, B8


_RUNNER = None


def get_runner():
    """Build the bass program once and wrap it in a persistent jitted
    shard_map executable over the 8 cores. Returns
    (sharded_fn, in_names, out_names, out_avals, n_params)."""
    global _RUNNER
    if _RUNNER is not None:
        return _RUNNER
    import jax
    from jax.sharding import Mesh, PartitionSpec
    from jax.experimental.shard_map import shard_map
    from concourse import bass2jax

    nc = build_program()
    bass2jax.install_neuronx_cc_hook()

    partition_name = (nc.partition_id_tensor.name
                      if nc.partition_id_tensor else None)
    in_names, out_names, out_avals = [], [], []
    for alloc in nc.m.functions[0].allocations:
        if not isinstance(alloc, mybir.MemoryLocationSet):
            continue
        name = alloc.memorylocations[0].name
        if alloc.kind == "ExternalInput":
            if name != partition_name:
                in_names.append(name)
        elif alloc.kind == "ExternalOutput":
            out_names.append(name)
            out_avals.append(jax.core.ShapedArray(
                tuple(alloc.tensor_shape), mybir.dt.np(alloc.dtype)))
    n_params = len(in_names)
    all_names = in_names + out_names
    if partition_name is not None:
        all_names = all_names + [partition_name]
    donate = tuple(range(n_params, n_params + len(out_names)))

    def _body(*args):
        operands = list(args)
        if partition_name is not None:
            operands.append(bass2jax.partition_id_tensor())
        outs = bass2jax._bass_exec_p.bind(
            *operands,
            out_avals=tuple(out_avals),
            in_names=tuple(all_names),
            out_names=tuple(out_names),
            lowering_input_output_aliases=(),
            sim_require_finite=True,
            sim_require_nnan=True,
            nc=nc,
        )
        return tuple(outs)

    devices = jax.devices()[:M]
    mesh = Mesh(np.asarray(devices), ("core",))
    nin = n_params + len(out_names)
    sharded = jax.jit(
        shard_map(_body, mesh=mesh,
                  in_specs=(PartitionSpec("core"),) * nin,
                  out_specs=(PartitionSpec("core"),) * len(out_names),
                  check_rep=False),
        donate_argnums=donate, keep_unused=True)
    _RUNNER = (sharded, in_names, out_names, out_avals, n_params, mesh)
    return _RUNNER


def run_cores(in_maps):
    """Execute on the 8 cores from per-core input dicts; returns list of
    per-core output dicts."""
    sharded, in_names, out_names, out_avals, n_params, _ = get_runner()
    concat_in = [np.concatenate([in_maps[c][n] for c in range(M)], axis=0)
                 for n in in_names]
    concat_zeros = [np.zeros((M * a.shape[0], *a.shape[1:]), a.dtype)
                    for a in out_avals]
    out_arrs = sharded(*concat_in, *concat_zeros)
    return [
        {n: np.asarray(out_arrs[i]).reshape(M, *out_avals[i].shape)[c]
         for i, n in enumerate(out_names)}
        for c in range(M)
    ]


def host_in_maps(inputs):
    """Full inputs dict -> list of 8 per-core input dicts."""
    inputs = {k: np.asarray(v, np.float32) for k, v in inputs.items()}
    obs, done, h0, c0 = (inputs['obs'], inputs['done'], inputs['h0'],
                         inputs['c0'])
    conv1_w, conv1_b = inputs['conv1_w'], inputs['conv1_b']
    conv2_w, conv2_b = inputs['conv2_w'], inputs['conv2_b']
    conv3_w, conv3_b = inputs['conv3_w'], inputs['conv3_b']
    w_ih, w_hh, b_ih, b_hh = (inputs['w_ih'], inputs['w_hh'],
                              inputs['b_ih'], inputs['b_hh'])
    actor_w, actor_b = inputs['actor_w'], inputs['actor_b']
    critic_w, critic_b = inputs['critic_w'], inputs['critic_b']

    T1, T2f, T3f = build_toeplitz(conv1_w, conv2_w, conv3_w)
    WIH, WHH, BIAS, W8, B8 = build_lstm_weights(
        w_ih, w_hh, b_ih, b_hh, actor_w, actor_b, critic_w, critic_b)
    # conv biases are per-out-channel -> per-partition vectors in the
    # (channel, x) partition layouts used on device.
    CB1 = np.repeat(conv1_b, 6)[:, None].astype(np.float32)          # [96,1]
    CB2 = np.stack([np.repeat(conv2_b[16 * h: 16 * (h + 1)], 5)
                    for h in range(2)], 1).astype(np.float32)        # [80,2]
    CB3 = np.stack([np.repeat(conv3_b[32 * m: 32 * (m + 1)], 4)
                    for m in range(2)], 1).astype(np.float32)        # [128,2]
    shared = dict(eye=np.eye(128, dtype=np.float32),
                  ones1=np.ones((1, 128), np.float32),
                  T1=T1, T2f=T2f, T3f=T3f, WIH=WIH, WHH=WHH, BIAS=BIAS,
                  W8=W8, B8=B8, CB1=CB1, CB2=CB2, CB3=CB3)

    obs_r = obs.reshape(T, B, 147)
    done_r = done.reshape(T, B)
    in_maps = []
    for m in range(M):
        sl = slice(m * BL, (m + 1) * BL)
        in_maps.append(dict(
            obs_c=np.ascontiguousarray(obs_r[:, sl].reshape(N_LOC, 147)),
            doneT=np.ascontiguousarray(done_r[:, sl].reshape(1, N_LOC)),
            h0T=np.ascontiguousarray(h0[0, sl].T),
            c0T=np.ascontiguousarray(c0[0, sl].T),
            **shared))
    return in_maps


def kernel(obs, done, h0, c0,
           conv1_w, conv1_b, conv2_w, conv2_b, conv3_w, conv3_b,
           w_ih, w_hh, b_ih, b_hh, actor_w, actor_b, critic_w, critic_b):
    in_maps = host_in_maps(dict(
        obs=obs, done=done, h0=h0, c0=c0,
        conv1_w=conv1_w, conv1_b=conv1_b, conv2_w=conv2_w, conv2_b=conv2_b,
        conv3_w=conv3_w, conv3_b=conv3_b, w_ih=w_ih, w_hh=w_hh,
        b_ih=b_ih, b_hh=b_hh, actor_w=actor_w, actor_b=actor_b,
        critic_w=critic_w, critic_b=critic_b))
    results = run_cores(in_maps)

    out = np.zeros((T, B, 8), np.float32)
    hT = np.zeros((1, B, 128), np.float32)
    cT = np.zeros((1, B, 128), np.float32)
    for m in range(M):
        sl = slice(m * BL, (m + 1) * BL)
        out[:, sl] = results[m]["out8"].reshape(T, BL, 8)
        hT[0, sl] = results[m]["hT_o"]
        cT[0, sl] = results[m]["cT_o"]
    return out.reshape(T * B, 8), hT, cT


# revision 36
# speedup vs baseline: 521.4841x; 8.0926x over previous
"""Trainium2 Bass kernel for nn_MinigridPPOLSTMAgent.

Data-parallel over envs: B=256 split across 8 NeuronCores (32 envs each).
Per-core layout keeps features on SBUF partitions and images on the free
dim end-to-end:
  - obs tiles are PE-transposed into per-image-row feature chunks
  - conv1/2/3 are block-Toeplitz matmuls (host-built, zero-embedded)
  - the LSTM input projection is 8 accumulating K=128 matmuls per gate
  - the recurrence runs with gate weights as the stationary operand over
    [128 hid, 32 env] tiles; masks are PE-broadcast from the done flags
  - actor/critic heads + final transposes run on the PE
"""
import sys

sys.path.insert(0, '/opt/trn_rl_repo')

import numpy as np
import concourse.bass as bass
import concourse.mybir as mybir
import concourse.tile as tile
from concourse import bacc

T, B, HID, NA = 128, 256, 128, 7
M = 8
BL = B // M          # 32 envs per core
N_LOC = T * BL       # 4096 images per core
NBLK = 512           # images per bulk block (= 16 timesteps)
NBLOCKS = N_LOC // NBLK
F32 = mybir.dt.float32
F32R = mybir.dt.float32r
AF = mybir.ActivationFunctionType
ALU = mybir.AluOpType


# ---------------------------------------------------------------- program
def build_program():
    nc = bacc.Bacc("TRN2", target_bir_lowering=False, debug=False)

    def din(name, shape):
        return nc.dram_tensor(name, shape, F32, kind="ExternalInput")

    def dout(name, shape):
        return nc.dram_tensor(name, shape, F32, kind="ExternalOutput")

    obs_d = din("obs_c", [N_LOC, 147])
    done_d = din("doneT", [1, N_LOC])
    h0_d = din("h0T", [128, BL])
    c0_d = din("c0T", [128, BL])
    eye_d = din("eye", [128, 128])
    ones_d = din("ones1", [1, 128])
    t1_d = din("T1", [42, 96])
    t2_d = din("T2f", [96, 4 * 80])       # (dy,h) major on cols
    t3_d = din("T3f", [80, 8 * 128])      # ((dy,h,m)) major on cols
    wih_d = din("WIH", [128, 8 * 512])    # k major on cols
    whh_d = din("WHH", [128, 512])
    bias_d = din("BIAS", [128, 4])
    w8_d = din("W8", [128, 8])
    b8_d = din("B8", [8, 1])
    cb1_d = din("CB1", [96, 1])
    cb2_d = din("CB2", [80, 2])
    cb3_d = din("CB3", [128, 2])

    out8_d = dout("out8", [N_LOC, 8])
    ht_d = dout("hT_o", [BL, 128])
    ct_d = dout("cT_o", [BL, 128])

    from contextlib import ExitStack
    with tile.TileContext(nc) as tc, ExitStack() as es:
        sing = es.enter_context(tc.tile_pool(name="sing", bufs=1))
        big = es.enter_context(tc.tile_pool(name="big", bufs=1))
        obs_p = es.enter_context(tc.tile_pool(name="obs", bufs=4))
        x0_p = es.enter_context(tc.tile_pool(name="x0", bufs=5))
        x1_p = es.enter_context(tc.tile_pool(name="x1", bufs=1))
        x2_p = es.enter_context(tc.tile_pool(name="x2", bufs=1))
        x3_p = es.enter_context(tc.tile_pool(name="x3", bufs=1))
        rec_p = es.enter_context(tc.tile_pool(name="rec", bufs=3))
        cpool = es.enter_context(tc.tile_pool(name="cst", bufs=2))
        o8_p = es.enter_context(tc.tile_pool(name="o8", bufs=1))
        ps = es.enter_context(tc.tile_pool(name="ps", bufs=6, space="PSUM"))
        psg = es.enter_context(tc.tile_pool(name="psg", bufs=1, space="PSUM"))

        def load(dram, shape):
            t = sing.tile(shape, F32, tag=dram.name)
            nc.sync.dma_start(t[:], dram[:])
            return t

        gx = big.tile([128, 128 * N_LOC // BL], F32R, tag="gx")  # [128, 16384]
        msk = big.tile([128, N_LOC], F32, tag="msk")
        hbuf = big.tile([128, N_LOC], F32, tag="hbuf")

        eye = load(eye_d, [128, 128])
        ones1 = load(ones_d, [1, 128])
        whh = load(whh_d, [128, 512])
        bias = load(bias_d, [128, 4])
        w8 = load(w8_d, [128, 8])
        b8 = load(b8_d, [8, 1])

        def load_r(dram, shape, tag):
            # stage as fp32 inside hbuf (unused until the recurrence), then
            # DVE-copy to round into fp32r (the gpsimd cast-DMA path crashes
            # the exec unit on TRN2)
            s = hbuf[0:shape[0], 0:shape[1]]
            nc.sync.dma_start(s, dram[:])
            t = sing.tile(shape, F32R, tag=tag)
            nc.vector.tensor_copy(t[:], s)
            return t

        eye_r = sing.tile([128, 128], F32R, tag="eyer")
        nc.vector.tensor_copy(eye_r[:], eye[:])
        t1 = load_r(t1_d, [42, 96], "t1r")
        t2 = load_r(t2_d, [96, 320], "t2r")
        t3 = load_r(t3_d, [80, 1024], "t3r")
        wih = load_r(wih_d, [128, 4096], "wihr")
        cb1 = load(cb1_d, [96, 1])
        cb2 = load(cb2_d, [80, 2])
        cb3 = load(cb3_d, [128, 2])
        done_s = load(done_d, [1, N_LOC])
        h0s = load(h0_d, [128, BL])
        c0s = load(c0_d, [128, BL])

        # ---------------- bulk phase: convs + x-projection + masks
        for blk in range(NBLOCKS):
            i0 = blk * NBLK
            obs_t = []
            for half in range(NBLK // 128):
                ot = obs_p.tile([128, 147], F32, tag="obs")
                nc.sync.dma_start(
                    ot[:], obs_d[i0 + 128 * half: i0 + 128 * (half + 1), :])
                obs_t.append(ot)
            # transpose into per-y1 feature chunks
            x0t = []
            for y1 in range(6):
                xt = x0_p.tile([42, NBLK], F32R, tag="x0")
                for half in range(NBLK // 128):
                    pt = ps.tile([42, 128], F32, tag="bank")
                    nc.tensor.transpose(
                        pt[:], obs_t[half][:, 21 * y1: 21 * y1 + 42], eye[:])
                    nc.vector.tensor_copy(
                        xt[:, 128 * half: 128 * (half + 1)], pt[:])
                x0t.append(xt)
            # conv1 (evac on DVE: relu(x + bias))
            x1t = x1_p.tile([96, 6 * NBLK], F32R, tag="x1")
            for y1 in range(6):
                p1 = ps.tile([96, NBLK], F32, tag="bank")
                nc.tensor.matmul(p1[:], t1[:], x0t[y1][:], start=True, stop=True)
                nc.vector.tensor_scalar(
                    x1t[:, NBLK * y1: NBLK * (y1 + 1)], p1[:],
                    cb1[:, 0:1], 0.0, ALU.add, ALU.max)
            # conv2  (x2 col layout: (h,y2) blocks; evac on DVE)
            x2t = x2_p.tile([80, 10 * NBLK], F32R, tag="x2")
            for y2 in range(5):
                for h in range(2):
                    p2 = ps.tile([80, NBLK], F32, tag="bank")
                    for dy in range(2):
                        nc.tensor.matmul(
                            p2[:],
                            t2[:, 80 * (dy * 2 + h): 80 * (dy * 2 + h + 1)],
                            x1t[:, NBLK * (y2 + dy): NBLK * (y2 + dy + 1)],
                            start=(dy == 0), stop=(dy == 1))
                    c = h * 5 + y2
                    nc.vector.tensor_scalar(
                        x2t[:, NBLK * c: NBLK * (c + 1)], p2[:],
                        cb2[:, h: h + 1], 0.0, ALU.add, ALU.max)
            # conv3 (x3 chunk k = y3*2+m)
            x3t = x3_p.tile([128, 8 * NBLK], F32R, tag="x3")
            for y3 in range(4):
                for mm in range(2):
                    p3 = ps.tile([128, NBLK], F32, tag="bank")
                    i = 0
                    for dy in range(2):
                        for h in range(2):
                            w = (dy * 2 + h) * 2 + mm
                            c = h * 5 + (y3 + dy)
                            nc.tensor.matmul(
                                p3[:],
                                t3[:, 128 * w: 128 * (w + 1)],
                                x2t[:, NBLK * c: NBLK * (c + 1)],
                                start=(i == 0), stop=(i == 3))
                            i += 1
                    k = y3 * 2 + mm
                    nc.scalar.activation(
                        x3t[:, NBLK * k: NBLK * (k + 1)], p3[:], AF.Relu,
                        bias=cb3[:, mm: mm + 1])
            # x-projection per gate (gate cols order: i,f,o,g)
            gxv = gx[:].rearrange("p (t g b) -> p t g b", g=4, b=BL)
            for g in range(4):
                px = ps.tile([128, NBLK], F32, tag="bank")
                for k in range(8):
                    nc.tensor.matmul(
                        px[:],
                        wih[:, 512 * k + 128 * g: 512 * k + 128 * (g + 1)],
                        x3t[:, NBLK * k: NBLK * (k + 1)],
                        start=(k == 0), stop=(k == 7))
                tpb = NBLK // BL
                dest = gxv[:, tpb * blk: tpb * (blk + 1), g: g + 1, :]
                src = px[:].rearrange("p (t b) -> p t b", b=BL)[:, :, None, :]
                nc.scalar.activation(dest, src, AF.Identity,
                                     bias=bias[:, g: g + 1])
            # masks: broadcast (1-done) across partitions
            pm = ps.tile([128, NBLK], F32, tag="bank")
            nc.tensor.matmul(pm[:], ones1[:], done_s[0:1, i0: i0 + NBLK],
                             start=True, stop=True)
            nc.scalar.activation(msk[:, i0: i0 + NBLK], pm[:], AF.Identity,
                                 scale=-1.0, bias=1.0)

        # ---------------- recurrence: two interleaved 16-env chains
        HB = BL // 2
        gxv4 = gx[:].rearrange("p (t g b) -> p t g b", g=4, b=BL)
        h_prev = [h0s[:, HB * s: HB * (s + 1)] for s in range(2)]
        c_prev = [c0s[:, HB * s: HB * (s + 1)] for s in range(2)]
        def step_stages(t, s):
            """Yield one closure per pipeline stage of (step t, chain s)."""
            mt = msk[:, BL * t + HB * s: BL * t + HB * (s + 1)]
            st = {}

            def s_hm():
                st['hm'] = rec_p.tile([128, HB], F32, tag=f"hm{s}", name=f"hm{s}_{t}")
                nc.vector.tensor_mul(st['hm'][:], h_prev[s], mt)
                st['cm'] = rec_p.tile([128, HB], F32, tag=f"cm{s}", name=f"cm{s}_{t}")
                nc.gpsimd.tensor_mul(st['cm'][:], c_prev[s], mt)

            def s_mm():
                # gx (bias folded, g-gate pre-doubled) is injected into PSUM
                # via an identity matmul; the 4 gate matmuls accumulate on it.
                st['gp'] = psg.tile([128, 4 * HB], F32, tag=f"gp{s}", name=f"gp{s}_{t}")
                nc.tensor.matmul(
                    st['gp'][:], eye_r[:],
                    gxv4[:, t, :, HB * s: HB * (s + 1)],
                    start=True, stop=False, skip_group_check=True)
                for g in range(4):
                    nc.tensor.matmul(
                        st['gp'][:, HB * g: HB * (g + 1)],
                        whh[:, 128 * g: 128 * (g + 1)], st['hm'][:],
                        start=False, stop=True, skip_group_check=True)

            def s_act():
                # gates (i,f,o,g); g-gate args doubled on the host so
                # tanh(x) = 2*sigmoid(2x) - 1 shares this single Sigmoid.
                st['sg'] = rec_p.tile([128, 4 * HB], F32, tag=f"sg{s}", name=f"sg{s}_{t}")
                nc.scalar.activation(st['sg'][:], st['gp'][:], AF.Sigmoid)

            def s_cupd():
                sg = st['sg']
                fc = rec_p.tile([128, HB], F32, tag=f"fc{s}", name=f"fc{s}_{t}")
                nc.vector.tensor_mul(fc[:], sg[:, HB:2 * HB], st['cm'][:])
                x1 = rec_p.tile([128, HB], F32, tag=f"ig{s}", name=f"x1{s}_{t}")
                nc.vector.scalar_tensor_tensor(
                    x1[:], sg[:, 3 * HB:4 * HB], 2.0, sg[:, 0:HB],
                    ALU.mult, ALU.mult)
                x2 = rec_p.tile([128, HB], F32, tag=f"x2{s}", name=f"x2{s}_{t}")
                nc.vector.tensor_sub(x2[:], x1[:], sg[:, 0:HB])
                st['cn'] = cpool.tile([128, HB], F32, tag=f"c{s}", name=f"cn{s}_{t}")
                nc.vector.tensor_add(st['cn'][:], x2[:], fc[:])

            def s_tanh():
                st['tc'] = rec_p.tile([128, HB], F32, tag=f"tc{s}", name=f"tc{s}_{t}")
                nc.scalar.activation(st['tc'][:], st['cn'][:], AF.Tanh)

            def s_h():
                nc.vector.tensor_mul(
                    hbuf[:, BL * t + HB * s: BL * t + HB * (s + 1)],
                    st['sg'][:, 2 * HB:3 * HB], st['tc'][:])
                h_prev[s] = hbuf[:, BL * t + HB * s: BL * t + HB * (s + 1)]
                c_prev[s] = st['cn'][:]

            return [s_hm, s_mm, s_act, s_cupd, s_tanh, s_h]

        for t in range(T):
            for st0, st1 in zip(step_stages(t, 0), step_stages(t, 1)):
                st0()
                st1()
        # concatenate per-chain final c into one tile for the transpose
        c_cat = rec_p.tile([128, BL], F32, tag="ccat")
        for s in range(2):
            nc.vector.tensor_copy(c_cat[:, HB * s: HB * (s + 1)], c_prev[s])
        c_prev = c_cat[:]
        h_prev = hbuf[:, BL * (T - 1): BL * T]

        # ---------------- heads + output transposes
        for j in range(8):
            p8 = ps.tile([8, 512], F32, tag="bank")
            nc.tensor.matmul(p8[:], w8[:], hbuf[:, 512 * j: 512 * (j + 1)],
                             start=True, stop=True)
            o8t = o8_p.tile([8, 512], F32, tag="o8")
            nc.scalar.activation(o8t[:], p8[:], AF.Identity, bias=b8[:, 0:1])
            for q in range(4):
                pt8 = ps.tile([128, 8], F32, tag="bank")
                nc.tensor.transpose(pt8[:], o8t[:, 128 * q: 128 * (q + 1)],
                                    eye[0:8, 0:8])
                ot8 = o8_p.tile([128, 8], F32, tag="ot8")
                nc.vector.tensor_copy(ot8[:], pt8[:])
                r0 = 512 * j + 128 * q
                nc.sync.dma_start(out8_d[r0: r0 + 128, :], ot8[:])
        # final h/c (transposed to [BL, 128])
        for src, dst in ((h_prev, ht_d), (c_prev, ct_d)):  # [128, BL] APs
            phc = ps.tile([BL, 128], F32, tag="bank")
            nc.tensor.transpose(phc[:], src, eye[:])
            hct = o8_p.tile([BL, 128], F32, tag="hct")
            nc.vector.tensor_copy(hct[:], phc[:])
            nc.sync.dma_start(dst[:], hct[:])

    nc.compile()
    return nc


# ---------------------------------------------------------------- host side
def build_toeplitz(conv1_w, conv2_w, conv3_w):
    T1 = np.zeros((42, 96), np.float32)
    for c1 in range(16):
        for x1 in range(6):
            for dy in range(2):
                for dx in range(2):
                    for c in range(3):
                        T1[21 * dy + 3 * (x1 + dx) + c, c1 * 6 + x1] = \
                            conv1_w[c1, c, dy, dx]
    T2 = np.zeros((2, 2, 96, 80), np.float32)
    for h in range(2):
        for c2p in range(16):
            for x2 in range(5):
                for dy in range(2):
                    for dx in range(2):
                        for c1 in range(16):
                            T2[dy, h, c1 * 6 + (x2 + dx), c2p * 5 + x2] = \
                                conv2_w[16 * h + c2p, c1, dy, dx]
    T3 = np.zeros((2, 2, 2, 80, 128), np.float32)
    for mm in range(2):
        for c3p in range(32):
            for x3 in range(4):
                for dy in range(2):
                    for dx in range(2):
                        for h in range(2):
                            for c2p in range(16):
                                T3[dy, h, mm, c2p * 5 + (x3 + dx), c3p * 4 + x3] = \
                                    conv3_w[32 * mm + c3p, 16 * h + c2p, dy, dx]
    # flatten to the on-device column layouts
    T2f = np.zeros((96, 4 * 80), np.float32)
    for dy in range(2):
        for h in range(2):
            T2f[:, 80 * (dy * 2 + h): 80 * (dy * 2 + h + 1)] = T2[dy, h]
    T3f = np.zeros((80, 8 * 128), np.float32)
    for dy in range(2):
        for h in range(2):
            for mm in range(2):
                w = (dy * 2 + h) * 2 + mm
                T3f[:, 128 * w: 128 * (w + 1)] = T3[dy, h, mm]
    return T1, T2f, T3f


def build_lstm_weights(w_ih, w_hh, b_ih, b_hh, actor_w, actor_b,
                       critic_w, critic_b):
    gperm = np.concatenate([np.arange(0, 128), np.arange(128, 256),
                            np.arange(384, 512), np.arange(256, 384)])
    fperm = np.zeros(1024, np.int64)
    idx = 0
    for y3 in range(4):
        for mm in range(2):
            for c3p in range(32):
                for x3 in range(4):
                    fperm[idx] = (c3p + 32 * mm) * 16 + y3 * 4 + x3
                    idx += 1
    wp = w_ih[gperm][:, fperm].T            # [1024, 512]
    WIH = np.zeros((128, 8 * 512), np.float32)
    for k in range(8):
        WIH[:, 512 * k: 512 * (k + 1)] = wp[128 * k: 128 * (k + 1)]
    WHH = np.ascontiguousarray(w_hh[gperm].T, np.float32)
    BIAS = np.ascontiguousarray((b_ih + b_hh)[gperm].reshape(4, 128).T,
                                np.float32)
    # tanh(x) = 2*sigmoid(2x) - 1: pre-double the g-gate arguments so the
    # device computes all four gates with a single Sigmoid op.
    for k in range(8):
        WIH[:, 512 * k + 384: 512 * (k + 1)] *= 2.0
    WHH[:, 384:512] *= 2.0
    BIAS[:, 3] *= 2.0
    W8 = np.ascontiguousarray(np.concatenate([actor_w, critic_w], 0).T,
                              np.float32)
    B8 = np.ascontiguousarray(
        np.concatenate([actor_b, critic_b])[:, None], np.float32)
    return WIH, WHH, BIAS, W8, B8


_RUNNER = None


def get_runner():
    """Build the bass program once and wrap it in a persistent jitted
    shard_map executable over the 8 cores. Returns
    (sharded_fn, in_names, out_names, out_avals, n_params)."""
    global _RUNNER
    if _RUNNER is not None:
        return _RUNNER
    import jax
    from jax.sharding import Mesh, PartitionSpec
    from jax.experimental.shard_map import shard_map
    from concourse import bass2jax

    nc = build_program()
    bass2jax.install_neuronx_cc_hook()

    partition_name = (nc.partition_id_tensor.name
                      if nc.partition_id_tensor else None)
    in_names, out_names, out_avals = [], [], []
    for alloc in nc.m.functions[0].allocations:
        if not isinstance(alloc, mybir.MemoryLocationSet):
            continue
        name = alloc.memorylocations[0].name
        if alloc.kind == "ExternalInput":
            if name != partition_name:
                in_names.append(name)
        elif alloc.kind == "ExternalOutput":
            out_names.append(name)
            out_avals.append(jax.core.ShapedArray(
                tuple(alloc.tensor_shape), mybir.dt.np(alloc.dtype)))
    n_params = len(in_names)
    all_names = list(in_names)
    if partition_name is not None:
        all_names = all_names + [partition_name]

    def _body(*args):
        operands = list(args)
        if partition_name is not None:
            operands.append(bass2jax.partition_id_tensor())
        outs = bass2jax._bass_exec_p.bind(
            *operands,
            out_avals=tuple(out_avals),
            in_names=tuple(all_names),
            out_names=tuple(out_names),
            lowering_input_output_aliases=(),
            sim_require_finite=True,
            sim_require_nnan=True,
            nc=nc,
        )
        return tuple(outs)

    devices = jax.devices()[:M]
    mesh = Mesh(np.asarray(devices), ("core",))
    sharded = jax.jit(
        shard_map(_body, mesh=mesh,
                  in_specs=(PartitionSpec("core"),) * n_params,
                  out_specs=(PartitionSpec("core"),) * len(out_names),
                  check_rep=False),
        keep_unused=True)

    h0_idx = in_names.index("h0T")

    def _chain(k):
        def body_k(*args):
            args = list(args)
            outs = None
            import jax.numpy as jnp
            for _ in range(k):
                outs = _body(*args)
                # feed hT_o back into h0T to serialize iterations
                args[h0_idx] = jnp.reshape(outs[out_names.index("hT_o")],
                                           (128, BL))
            return outs
        return jax.jit(
            shard_map(body_k, mesh=mesh,
                      in_specs=(PartitionSpec("core"),) * n_params,
                      out_specs=(PartitionSpec("core"),) * len(out_names),
                      check_rep=False),
            keep_unused=True)

    _RUNNER = (sharded, in_names, out_names, out_avals, n_params, mesh,
               _chain)
    return _RUNNER


def run_cores(in_maps):
    """Execute on the 8 cores from per-core input dicts; returns list of
    per-core output dicts."""
    sharded, in_names, out_names, out_avals, n_params, _, _ = get_runner()
    concat_in = [np.concatenate([in_maps[c][n] for c in range(M)], axis=0)
                 for n in in_names]
    out_arrs = sharded(*concat_in)
    return [
        {n: np.asarray(out_arrs[i]).reshape(M, *out_avals[i].shape)[c]
         for i, n in enumerate(out_names)}
        for c in range(M)
    ]


def host_in_maps(inputs):
    """Full inputs dict -> list of 8 per-core input dicts."""
    inputs = {k: np.asarray(v, np.float32) for k, v in inputs.items()}
    obs, done, h0, c0 = (inputs['obs'], inputs['done'], inputs['h0'],
                         inputs['c0'])
    conv1_w, conv1_b = inputs['conv1_w'], inputs['conv1_b']
    conv2_w, conv2_b = inputs['conv2_w'], inputs['conv2_b']
    conv3_w, conv3_b = inputs['conv3_w'], inputs['conv3_b']
    w_ih, w_hh, b_ih, b_hh = (inputs['w_ih'], inputs['w_hh'],
                              inputs['b_ih'], inputs['b_hh'])
    actor_w, actor_b = inputs['actor_w'], inputs['actor_b']
    critic_w, critic_b = inputs['critic_w'], inputs['critic_b']

    T1, T2f, T3f = build_toeplitz(conv1_w, conv2_w, conv3_w)
    WIH, WHH, BIAS, W8, B8 = build_lstm_weights(
        w_ih, w_hh, b_ih, b_hh, actor_w, actor_b, critic_w, critic_b)
    # conv biases are per-out-channel -> per-partition vectors in the
    # (channel, x) partition layouts used on device.
    CB1 = np.repeat(conv1_b, 6)[:, None].astype(np.float32)          # [96,1]
    CB2 = np.stack([np.repeat(conv2_b[16 * h: 16 * (h + 1)], 5)
                    for h in range(2)], 1).astype(np.float32)        # [80,2]
    CB3 = np.stack([np.repeat(conv3_b[32 * m: 32 * (m + 1)], 4)
                    for m in range(2)], 1).astype(np.float32)        # [128,2]
    shared = dict(eye=np.eye(128, dtype=np.float32),
                  ones1=np.ones((1, 128), np.float32),
                  T1=T1, T2f=T2f, T3f=T3f, WIH=WIH, WHH=WHH, BIAS=BIAS,
                  W8=W8, B8=B8, CB1=CB1, CB2=CB2, CB3=CB3)

    obs_r = obs.reshape(T, B, 147)
    done_r = done.reshape(T, B)
    in_maps = []
    for m in range(M):
        sl = slice(m * BL, (m + 1) * BL)
        in_maps.append(dict(
            obs_c=np.ascontiguousarray(obs_r[:, sl].reshape(N_LOC, 147)),
            doneT=np.ascontiguousarray(done_r[:, sl].reshape(1, N_LOC)),
            h0T=np.ascontiguousarray(h0[0, sl].T),
            c0T=np.ascontiguousarray(c0[0, sl].T),
            **shared))
    return in_maps


def kernel(obs, done, h0, c0,
           conv1_w, conv1_b, conv2_w, conv2_b, conv3_w, conv3_b,
           w_ih, w_hh, b_ih, b_hh, actor_w, actor_b, critic_w, critic_b):
    in_maps = host_in_maps(dict(
        obs=obs, done=done, h0=h0, c0=c0,
        conv1_w=conv1_w, conv1_b=conv1_b, conv2_w=conv2_w, conv2_b=conv2_b,
        conv3_w=conv3_w, conv3_b=conv3_b, w_ih=w_ih, w_hh=w_hh,
        b_ih=b_ih, b_hh=b_hh, actor_w=actor_w, actor_b=actor_b,
        critic_w=critic_w, critic_b=critic_b))
    results = run_cores(in_maps)

    out = np.zeros((T, B, 8), np.float32)
    hT = np.zeros((1, B, 128), np.float32)
    cT = np.zeros((1, B, 128), np.float32)
    for m in range(M):
        sl = slice(m * BL, (m + 1) * BL)
        out[:, sl] = results[m]["out8"].reshape(T, BL, 8)
        hT[0, sl] = results[m]["hT_o"]
        cT[0, sl] = results[m]["cT_o"]
    return out.reshape(T * B, 8), hT, cT
